# revision 31
# baseline (speedup 1.0000x reference)
"""Trainium2 Bass kernel for NeuralClusteringAttention.

Problem: B=4, T=1024, D=512, C=8 clusters, H=8 heads, fp32.
Reference: per-token cluster assignment (argmax of a linear projection), then
for each cluster c: full MHA over X*mask_c, output masked and summed over c.

Key algebraic collapse (headroom source): since every token belongs to exactly
one cluster and in_proj bias is zero, the C-pass reference reduces to ONE
masked-attention pass:
    out_i = Wout @ (sum_{j: c_j=c_i} e^{s_ij} v_j) / (sum_{j: c_j=c_i} e^{s_ij}
            + (T - n_{c_i})) + bout
where s_ij = q_i.k_j/sqrt(hd). The (T - n_c) term accounts for the e^0=1
contributions of masked keys in the reference softmax (masked scores are 0,
not -inf).

Implementation tricks:
- Cluster masking is folded into the QK^T contraction: it is augmented with
  sqrt(lambda)*one_hot(cluster) rows so intra-cluster pairs get +lambda;
  exp(s/8 + lambda/8*(E-1)) suppresses cross-cluster pairs to ~e^-24.
- Tokens are SORTED by cluster on the host, so attention only computes the
  near-block-diagonal tiles (the lambda-mask handles block-boundary overlap
  exactly). The per-batch layouts share one canonical slot structure so a
  single SPMD program serves all cores.
- The softmax denominator is produced by the PV matmul itself: V is augmented
  with 64 ones-columns (mass lands replicated on out-partitions 64..127), and
  one zero-X "denominator token" per cluster carries weight (T - n_c) in its
  ones-columns, so no separate correction pass is needed.
- Matmul operands use dtype float32r (TF32-like fast PE path, ~4x fp32).

Sharding: 8 cores = (4 batches) x (2 head-halves of 4 heads each). Each core
computes QKV projections for its 4 heads, masked attention, and a partial
output projection; host sums the two partials per batch and adds bout.
"""

import numpy as np

import concourse.bacc as bacc
import concourse.bass as bass
import concourse.mybir as mybir
import concourse.tile as tile
from concourse.bass_utils import run_bass_kernel_spmd

B, T, D, C, H = 4, 1024, 512, 8, 8
HD = D // H          # 64
LH = 4               # local heads per core
LAMBDA = 256.0       # cluster-mask additive bias (lambda/8 = 32 in exp domain)
SQL = 16.0           # sqrt(LAMBDA)
F32 = mybir.dt.float32
MMDT = mybir.dt.float32r  # matmul operand dtype (float32r = fast PE path)
CHUNK = 256          # attention i-chunk width (psum: [128, 4, 256] = 2 banks)


def make_schedule(assign_all):
    """Canonical cluster-slot layout shared by all batches (single SPMD prog).

    Returns (T2, units, denom_cols, slot_of, As, Ws):
      units: list of (c0, c1, jb0, jb1) attention work items
      denom_cols: canonical column of each slot's denominator token
    """
    counts = np.stack([np.bincount(a, minlength=C) for a in assign_all])  # [B,C]
    order = np.argsort(-counts, axis=1, kind="stable")  # [B, C] slot -> cluster
    sizes_sorted = -np.sort(-counts, axis=1)
    Ws = sizes_sorted.max(axis=0) + 1  # +1 denominator token per slot
    As = np.zeros(C, np.int64)
    As[1:] = np.cumsum(Ws)[:-1]
    used = int(As[-1] + Ws[-1])
    T2 = ((used + 127) // 128) * 128
    units = []
    for c0 in range(0, used, CHUNK):
        # float32r matmuls need even moving-dim; round widths to 4 columns
        # (pad columns hold zeros / inert masks, outputs there are dropped)
        c1 = min(c0 + CHUNK, ((used + 3) // 4) * 4)
        s_lo = int(np.searchsorted(As, c0, "right") - 1)
        s_hi = int(np.searchsorted(As, c1 - 1, "right") - 1)
        jb0 = int(As[s_lo]) // 128
        jb1 = (int(As[s_hi] + Ws[s_hi]) + 127) // 128
        jb1 = min(jb1, T2 // 128)
        units.append((c0, c1, jb0, jb1))
    return T2, units, [int(a) for a in As], order, Ws


def _kernel_body(tc, T2, units, denom_cols):
    nc = tc.nc
    NB = T2 // 128
    NSC = [(o, min(o + 512, T2)) for o in range(0, T2, 512)]

    xt_d = nc.dram_tensor("xt", [D, T2], F32, kind="ExternalInput").ap()
    wqkv_d = nc.dram_tensor("wqkv", [D, 3 * 256], F32, kind="ExternalInput").ap()
    wo_d = nc.dram_tensor("wo", [256, D], F32, kind="ExternalInput").ap()
    msl_d = nc.dram_tensor("msl", [C, T2], F32, kind="ExternalInput").ap()
    vp_d = nc.dram_tensor("vpatch", [C, LH, 64], F32, kind="ExternalInput").ap()
    out_d = nc.dram_tensor("outT", [D, T2], F32, kind="ExternalOutput").ap()

    with (
        tc.tile_pool(name="const", bufs=1) as const,
        tc.tile_pool(name="p1", bufs=4) as ppool,
        tc.tile_pool(name="rb", bufs=2) as rbpool,
        tc.tile_pool(name="acc", bufs=2, space="PSUM") as acc,
        tc.tile_pool(name="sps", bufs=2, space="PSUM") as sps,
    ):
        # ---- persistent SBUF tiles + input DMAs ----
        wqkv = const.tile([128, D // 128, 3 * 256], MMDT)
        wqkv_r = wqkv_d.rearrange("(c p) n -> p c n", p=128).bitcast(MMDT)
        for dc in range(D // 128):
            nc.sync.dma_start(wqkv[:, dc, :], wqkv_r[:, dc, :])
        xt = const.tile([128, D // 128, T2], MMDT)
        xt_r = xt_d.rearrange("(c p) t -> p c t", p=128).bitcast(MMDT)
        for n0, n1 in NSC:
            for dc in range(D // 128):
                nc.sync.dma_start(xt[:, dc, n0:n1], xt_r[:, dc, n0:n1])
        wo = const.tile([128, 2, D], MMDT)
        nc.sync.dma_start(
            wo[:], wo_d.rearrange("(c p) n -> p c n", p=128).bitcast(MMDT)
        )
        qtm = const.tile([72, LH, T2], MMDT)
        ktm = const.tile([72, LH, T2], MMDT)
        for h in range(LH):
            nc.sync.dma_start(qtm[64:72, h, :], msl_d[:, :].bitcast(MMDT))
            nc.sync.dma_start(ktm[64:72, h, :], msl_d[:, :].bitcast(MMDT))
        # V augmented with 64 ones-columns (softmax mass on partitions 64..127);
        # denominator-token rows carry (T - n_c) instead of 1.
        v_sb = const.tile([128, NB, LH, 128], MMDT)
        nc.vector.memset(v_sb[:, :, :, 64:128].bitcast(F32), 1.0)
        for s, dc_col in enumerate(denom_cols):
            nc.sync.dma_start(
                v_sb[dc_col % 128 : dc_col % 128 + 1, dc_col // 128, :, 64:128],
                vp_d[s : s + 1, :, :].bitcast(MMDT),
            )
        biasm = const.tile([128, 1], F32)
        nc.vector.memset(biasm[:, :], -LAMBDA / 8.0)
        o_sc = const.tile([128, 2, T2], MMDT)
        used = units[-1][1]
        if used < T2:
            nc.vector.memset(o_sc[:, :, used:T2].bitcast(F32), 0.0)

        # ---- QKV projections (head pairs, transposed layouts) ----
        for hp in range(2):
            for w_off, dst in ((0, qtm), (256, ktm)):
                for n0, n1 in NSC:
                    ps = sps.tile([128, 512], F32, tag="sps")
                    for dc in range(D // 128):
                        nc.tensor.matmul(
                            ps[:, : n1 - n0],
                            lhsT=wqkv[:, dc, w_off + hp * 128 : w_off + (hp + 1) * 128],
                            rhs=xt[:, dc, n0:n1],
                            start=(dc == 0),
                            stop=(dc == D // 128 - 1),
                        )
                    # even head aligned; odd head cross-half (legal 64-wide DVE)
                    nc.vector.tensor_copy(dst[0:64, 2 * hp, n0:n1], ps[0:64, : n1 - n0])
                    nc.vector.tensor_copy(
                        dst[0:64, 2 * hp + 1, n0:n1], ps[64:128, : n1 - n0]
                    )

        # ---- V projection in natural [token, dim] layout, all 4 heads ----
        for tb in range(NB):
            psv = sps.tile([128, 256], F32, tag="sps")
            for dc in range(D // 128):
                nc.tensor.matmul(
                    psv[:, 0:256],
                    lhsT=xt[:, dc, tb * 128 : (tb + 1) * 128],
                    rhs=wqkv[:, dc, 512:768],
                    start=(dc == 0),
                    stop=(dc == D // 128 - 1),
                )
            nc.scalar.copy(
                v_sb[:, tb, :, 0:64],
                psv[:, 0:256].rearrange("p (h d) -> p h d", h=4),
            )

        # ---- block-sparse masked attention (4 heads batched per unit) ----
        # PSUM start=True zeroes a whole 2KB bank, so the two heads sharing a
        # bank form ONE accumulation group: even head starts, odd head stops.
        for ui, (c0, c1, jb0, jb1) in enumerate(units):
            W = c1 - c0
            # fixed 256 stride keeps head regions at exact half-bank offsets
            oaug = acc.tile([128, LH, CHUNK], F32, tag="acc", name=f"oaug{ui}")
            for jb in range(jb0, jb1):
                s_ps = sps.tile(
                    [128, LH, CHUNK], F32, tag="sps", name=f"sps{ui}_{jb}"
                )
                for h in range(LH):
                    nc.tensor.matmul(
                        s_ps[:, h, 0:W],
                        lhsT=ktm[0:72, h, jb * 128 : (jb + 1) * 128],
                        rhs=qtm[0:72, h, c0:c1],
                        start=(h % 2 == 0),
                        stop=(h % 2 == 1),
                    )
                p1 = ppool.tile(
                    [128, LH, CHUNK], MMDT, tag="p1", name=f"p1_{ui}_{jb}"
                )
                nc.scalar.activation(
                    p1[:, :, 0:W],
                    s_ps[:, :, 0:W],
                    mybir.ActivationFunctionType.Exp,
                    bias=biasm[:, :],
                    scale=0.125,
                )
                for h in range(LH):
                    nc.tensor.matmul(
                        oaug[:, h, 0:W],
                        lhsT=v_sb[:, jb, h, 0:128],
                        rhs=p1[:, h, 0:W],
                        start=(h % 2 == 0 and jb == jb0),
                        stop=(h % 2 == 1 and jb == jb1 - 1),
                    )
            recip = rbpool.tile([64, LH, CHUNK], F32, tag="recip", name=f"rc{ui}")
            nc.vector.reciprocal(recip[:, :, 0:W], oaug[64:128, :, 0:W])
            for h in range(LH):
                nc.vector.tensor_mul(
                    o_sc[64 * (h % 2) : 64 * (h % 2) + 64, h // 2, c0:c1],
                    oaug[0:64, h, 0:W],
                    recip[:, h, 0:W],
                )



        # ---- output projection (partial over this core's heads) ----
        for doc in range(4):
            for n0, n1 in NSC:
                po = sps.tile([128, 512], F32, tag="sps", name=f"po{doc}_{n0}")
                for dhc in range(2):
                    nc.tensor.matmul(
                        po[:, : n1 - n0],
                        lhsT=wo[:, dhc, doc * 128 : (doc + 1) * 128],
                        rhs=o_sc[:, dhc, n0:n1],
                        start=(dhc == 0),
                        stop=(dhc == 1),
                    )
                ost = rbpool.tile([128, 512], F32, tag="ost", name=f"ost{doc}_{n0}")
                nc.scalar.copy(ost[:, : n1 - n0], po[:, : n1 - n0])
                nc.sync.dma_start(
                    out_d[doc * 128 : (doc + 1) * 128, n0:n1], ost[:, : n1 - n0]
                )


def build_nc(T2, units, denom_cols):
    nc = bacc.Bacc("TRN2", target_bir_lowering=False, debug=False, num_devices=8)
    with tile.TileContext(nc) as tc:
        _kernel_body(tc, T2, units, denom_cols)
    nc.compile()
    return nc


def prepare(X, Wc, bc, Win, Wout):
    """Host-side clustering, canonical layout, and per-core input maps."""
    X = np.asarray(X, np.float32)
    Wc = np.asarray(Wc, np.float32)
    bc = np.asarray(bc, np.float32)
    Win = np.asarray(Win, np.float32)
    Wout = np.asarray(Wout, np.float32)

    assign_all = np.stack(
        [(X[b] @ Wc.T + bc).argmax(-1) for b in range(B)]
    )  # [B, T]
    T2, units, denom_cols, order, Ws = make_schedule(assign_all)

    per_batch = []
    poss = []
    for b in range(B):
        a = assign_all[b]
        X2 = np.zeros((T2, D), np.float32)
        cid = np.full(T2, -1, np.int64)  # cluster id per column (-1 = pad)
        pos = np.empty(T, np.int64)  # original token -> column
        patch = np.empty((C, LH, 64), np.float32)
        for s in range(C):
            c = order[b, s]
            toks = np.nonzero(a == c)[0]
            n = len(toks)
            A = denom_cols[s]
            cid[A] = c  # denominator token
            patch[s] = float(T - n)
            cid[A + 1 : A + 1 + n] = c
            X2[A + 1 : A + 1 + n] = X[b, toks]
            pos[toks] = np.arange(A + 1, A + 1 + n)
        msl = SQL * (cid[None, :] == np.arange(C)[:, None]).astype(np.float32)
        per_batch.append(
            {
                "xt": np.ascontiguousarray(X2.T),
                "msl": np.ascontiguousarray(msl),
                "vpatch": patch,
            }
        )
        poss.append(pos)

    per_half = []
    for hh in range(2):
        r = slice(hh * 256, (hh + 1) * 256)
        wqkv = np.concatenate(
            [Win[0:D][r].T, Win[D : 2 * D][r].T, Win[2 * D :][r].T], axis=1
        )
        per_half.append(
            {
                "wqkv": np.ascontiguousarray(wqkv),
                "wo": np.ascontiguousarray(Wout[:, r].T),
            }
        )

    in_maps = [dict(per_batch[g // 2], **per_half[g % 2]) for g in range(8)]
    return (T2, units, denom_cols), in_maps, poss


_NC_CACHE = {}


def kernel(X, Wc, bc, Win, bin_, Wout, bout):
    assert not np.any(np.asarray(bin_)), "kernel assumes zero in_proj bias"
    sched, in_maps, poss = prepare(X, Wc, bc, Win, Wout)
    key = (sched[0], tuple(sched[1]), tuple(sched[2]))
    if key not in _NC_CACHE:
        _NC_CACHE[key] = build_nc(*sched)
    nc = _NC_CACHE[key]
    res = run_bass_kernel_spmd(nc, in_maps, core_ids=list(range(8)))
    outs = res.results
    bout = np.asarray(bout, np.float32)
    out = np.empty((B, T, D), np.float32)
    for b in range(B):
        full = outs[2 * b]["outT"] + outs[2 * b + 1]["outT"]  # [D, T2]
        out[b] = full.T[poss[b]] + bout
    return out


# revision 49
# speedup vs baseline: 1.1402x; 1.1402x over previous
"""Trainium2 Bass kernel for NeuralClusteringAttention.

Problem: B=4, T=1024, D=512, C=8 clusters, H=8 heads, fp32.
Reference: per-token cluster assignment (argmax of a linear projection), then
for each cluster c: full MHA over X*mask_c, output masked and summed over c.

Key algebraic collapse (headroom source): since every token belongs to exactly
one cluster and in_proj bias is zero, the C-pass reference reduces to ONE
masked-attention pass:
    out_i = Wout @ (sum_{j: c_j=c_i} e^{s_ij} v_j) / (sum_{j: c_j=c_i} e^{s_ij}
            + (T - n_{c_i})) + bout
where s_ij = q_i.k_j/sqrt(hd). The (T - n_c) term accounts for the e^0=1
contributions of masked keys in the reference softmax (masked scores are 0,
not -inf).

Implementation tricks:
- Cluster masking is folded into the QK^T contraction: it is augmented with
  sqrt(lambda)*one_hot(cluster) rows so intra-cluster pairs get +lambda;
  exp(s/8 + lambda/8*(E-1)) suppresses cross-cluster pairs to ~e^-24.
- Tokens are SORTED by cluster on the host, so attention only computes the
  near-block-diagonal tiles (the lambda-mask handles block-boundary overlap
  exactly). The per-batch layouts share one canonical slot structure so a
  single SPMD program serves all cores.
- The softmax denominator is produced by the PV matmul itself: V is augmented
  with 64 ones-columns (mass lands replicated on out-partitions 64..127), and
  one zero-X "denominator token" per cluster carries weight (T - n_c) in its
  ones-columns, so no separate correction pass is needed.
- Matmul operands use dtype float32r (TF32-like fast PE path, ~4x fp32).

Sharding: 8 cores = (4 batches) x (2 head-halves of 4 heads each). Each core
computes QKV projections for its 4 heads, masked attention, and a partial
output projection; host sums the two partials per batch and adds bout.
"""

import numpy as np

import concourse.bacc as bacc
import concourse.bass as bass
import concourse.mybir as mybir
import concourse.tile as tile
from concourse.bass_utils import run_bass_kernel_spmd

B, T, D, C, H = 4, 1024, 512, 8, 8
HD = D // H          # 64
LH = 4               # local heads per core
LAMBDA = 256.0       # cluster-mask additive bias (lambda/8 = 32 in exp domain)
SQL = 16.0           # sqrt(LAMBDA)
F32 = mybir.dt.float32
MMDT = mybir.dt.float32r  # matmul operand dtype (float32r = fast PE path)
CHUNK = 256          # attention i-chunk width (psum: [128, 4, 256] = 2 banks)


def make_schedule(assign_all):
    """Canonical cluster-slot layout shared by all batches (single SPMD prog).

    Returns (T2, units, denom_cols, slot_of, As, Ws):
      units: list of (c0, c1, jb0, jb1) attention work items
      denom_cols: canonical column of each slot's denominator token
    """
    counts = np.stack([np.bincount(a, minlength=C) for a in assign_all])  # [B,C]
    order = np.argsort(-counts, axis=1, kind="stable")  # [B, C] slot -> cluster
    sizes_sorted = -np.sort(-counts, axis=1)
    Ws = sizes_sorted.max(axis=0) + 1  # +1 denominator token per slot
    As = np.zeros(C, np.int64)
    As[1:] = np.cumsum(Ws)[:-1]
    used = int(As[-1] + Ws[-1])
    T2 = ((used + 127) // 128) * 128
    units = []
    for c0 in range(0, used, CHUNK):
        # float32r matmuls need even moving-dim; round widths to 4 columns
        # (pad columns hold zeros / inert masks, outputs there are dropped)
        c1 = min(c0 + CHUNK, ((used + 3) // 4) * 4)
        s_lo = int(np.searchsorted(As, c0, "right") - 1)
        s_hi = int(np.searchsorted(As, c1 - 1, "right") - 1)
        jb0 = int(As[s_lo]) // 128
        jb1 = (int(As[s_hi] + Ws[s_hi]) + 127) // 128
        jb1 = min(jb1, T2 // 128)
        units.append((c0, c1, jb0, jb1))
    return T2, units, [int(a) for a in As], order, Ws


def _kernel_body(tc, T2, units, denom_cols):
    nc = tc.nc
    NB = T2 // 128
    NSC = [(o, min(o + 512, T2)) for o in range(0, T2, 512)]

    xt_d = nc.dram_tensor("xt", [D, T2], F32, kind="ExternalInput").ap()
    wqkv_d = nc.dram_tensor("wqkv", [D, 3 * 256], F32, kind="ExternalInput").ap()
    wo_d = nc.dram_tensor("wo", [256, D], F32, kind="ExternalInput").ap()
    msl_d = nc.dram_tensor("msl", [C, T2], F32, kind="ExternalInput").ap()
    vp_d = nc.dram_tensor("vpatch", [C, LH, 64], F32, kind="ExternalInput").ap()
    out_d = nc.dram_tensor("outT", [D, T2], F32, kind="ExternalOutput").ap()

    with (
        tc.tile_pool(name="const", bufs=1) as const,
        tc.tile_pool(name="p1", bufs=8) as ppool,
        tc.tile_pool(name="rb", bufs=4) as rbpool,
        tc.tile_pool(name="acc", bufs=2, space="PSUM") as acc,
        tc.tile_pool(name="sps", bufs=2, space="PSUM") as sps,
    ):
        # ---- persistent SBUF tiles + input DMAs ----
        wqkv = const.tile([128, D // 128, 3 * 256], MMDT)
        wqkv_r = wqkv_d.rearrange("(c p) n -> p c n", p=128).bitcast(MMDT)
        xt = const.tile([128, D // 128, T2], MMDT)
        xt_r = xt_d.rearrange("(c p) t -> p c t", p=128).bitcast(MMDT)
        for dc in range(D // 128):
            nc.sync.dma_start(wqkv[:, dc, :], wqkv_r[:, dc, :])
            n0, n1 = NSC[0]
            nc.sync.dma_start(xt[:, dc, n0:n1], xt_r[:, dc, n0:n1])
        for n0, n1 in NSC[1:]:
            for dc in range(D // 128):
                nc.sync.dma_start(xt[:, dc, n0:n1], xt_r[:, dc, n0:n1])
        wo = const.tile([128, 2, D], MMDT)
        nc.sync.dma_start(
            wo[:], wo_d.rearrange("(c p) n -> p c n", p=128).bitcast(MMDT)
        )
        qtm = const.tile([72, LH, T2], MMDT)
        ktm = const.tile([72, LH, T2], MMDT)
        for h in range(LH):
            nc.sync.dma_start(qtm[64:72, h, :], msl_d[:, :].bitcast(MMDT))
            nc.sync.dma_start(ktm[64:72, h, :], msl_d[:, :].bitcast(MMDT))
        # V augmented with 64 ones-columns (softmax mass on partitions 64..127);
        # denominator-token rows carry (T - n_c) instead of 1.
        v_sb = const.tile([128, NB, LH, 128], MMDT)
        nc.vector.memset(v_sb[:, :, :, 64:128].bitcast(F32), 1.0)
        for s, dc_col in enumerate(denom_cols):
            nc.sync.dma_start(
                v_sb[dc_col % 128 : dc_col % 128 + 1, dc_col // 128, :, 64:128],
                vp_d[s : s + 1, :, :].bitcast(MMDT),
            )
        biasm = const.tile([128, 1], F32)
        nc.vector.memset(biasm[:, :], -LAMBDA / 8.0)
        o_sc = const.tile([128, 2, T2], MMDT)
        used = units[-1][1]
        if used < T2:
            nc.vector.memset(o_sc[:, :, used:T2].bitcast(F32), 0.0)

        # ---- QKV projections (head pairs, transposed layouts) ----
        for hp in range(2):
            for w_off, dst in ((0, qtm), (256, ktm)):
                for n0, n1 in NSC:
                    ps = sps.tile([128, 512], F32, tag="sps")
                    for dc in range(D // 128):
                        nc.tensor.matmul(
                            ps[:, : n1 - n0],
                            lhsT=wqkv[:, dc, w_off + hp * 128 : w_off + (hp + 1) * 128],
                            rhs=xt[:, dc, n0:n1],
                            start=(dc == 0),
                            stop=(dc == D // 128 - 1),
                        )
                    # even head aligned; odd head cross-half (legal 64-wide DVE)
                    nc.vector.tensor_copy(dst[0:64, 2 * hp, n0:n1], ps[0:64, : n1 - n0])
                    nc.vector.tensor_copy(
                        dst[0:64, 2 * hp + 1, n0:n1], ps[64:128, : n1 - n0]
                    )

        # ---- V projection in natural [token, dim] layout, all 4 heads ----
        for tb in range(NB):
            psv = sps.tile([128, 256], F32, tag="sps")
            for dc in range(D // 128):
                nc.tensor.matmul(
                    psv[:, 0:256],
                    lhsT=xt[:, dc, tb * 128 : (tb + 1) * 128],
                    rhs=wqkv[:, dc, 512:768],
                    start=(dc == 0),
                    stop=(dc == D // 128 - 1),
                )
            nc.scalar.copy(
                v_sb[:, tb, :, 0:64],
                psv[:, 0:256].rearrange("p (h d) -> p h d", h=4),
            )

        # ---- block-sparse masked attention (4 heads batched per unit) ----
        # PSUM start=True zeroes a whole 2KB bank, so the two heads sharing a
        # bank form ONE accumulation group: even head starts, odd head stops.
        for ui, (c0, c1, jb0, jb1) in enumerate(units):
            W = c1 - c0
            # fixed 256 stride keeps head regions at exact half-bank offsets
            oaug = acc.tile([128, LH, CHUNK], F32, tag="acc", name=f"oaug{ui}")
            for jb in range(jb0, jb1):
                s_ps = sps.tile(
                    [128, LH, CHUNK], F32, tag="sps", name=f"sps{ui}_{jb}"
                )
                for h in range(LH):
                    nc.tensor.matmul(
                        s_ps[:, h, 0:W],
                        lhsT=ktm[0:72, h, jb * 128 : (jb + 1) * 128],
                        rhs=qtm[0:72, h, c0:c1],
                        start=(h % 2 == 0),
                        stop=(h % 2 == 1),
                    )
                p1 = ppool.tile(
                    [128, LH, CHUNK], MMDT, tag="p1", name=f"p1_{ui}_{jb}"
                )
                nc.scalar.activation(
                    p1[:, :, 0:W],
                    s_ps[:, :, 0:W],
                    mybir.ActivationFunctionType.Exp,
                    bias=biasm[:, :],
                    scale=0.125,
                )
                for h in range(LH):
                    nc.tensor.matmul(
                        oaug[:, h, 0:W],
                        lhsT=v_sb[:, jb, h, 0:128],
                        rhs=p1[:, h, 0:W],
                        start=(h % 2 == 0 and jb == jb0),
                        stop=(h % 2 == 1 and jb == jb1 - 1),
                    )
            recip = rbpool.tile([64, LH, CHUNK], F32, tag="recip", name=f"rc{ui}")
            nc.vector.reciprocal(recip[:, :, 0:W], oaug[64:128, :, 0:W])
            for h in range(LH):
                nc.vector.tensor_mul(
                    o_sc[64 * (h % 2) : 64 * (h % 2) + 64, h // 2, c0:c1],
                    oaug[0:64, h, 0:W],
                    recip[:, h, 0:W],
                )



        # ---- output projection (partial over this core's heads) ----
        for doc in range(4):
            for n0, n1 in NSC:
                po = sps.tile([128, 512], F32, tag="sps", name=f"po{doc}_{n0}")
                for dhc in range(2):
                    nc.tensor.matmul(
                        po[:, : n1 - n0],
                        lhsT=wo[:, dhc, doc * 128 : (doc + 1) * 128],
                        rhs=o_sc[:, dhc, n0:n1],
                        start=(dhc == 0),
                        stop=(dhc == 1),
                    )
                ost = rbpool.tile([128, 512], F32, tag="ost", name=f"ost{doc}_{n0}")
                nc.scalar.copy(ost[:, : n1 - n0], po[:, : n1 - n0])
                nc.sync.dma_start(
                    out_d[doc * 128 : (doc + 1) * 128, n0:n1], ost[:, : n1 - n0]
                )


def build_nc(T2, units, denom_cols):
    nc = bacc.Bacc("TRN2", target_bir_lowering=False, debug=False, num_devices=8)
    with tile.TileContext(nc) as tc:
        _kernel_body(tc, T2, units, denom_cols)
    nc.compile()
    return nc


def prepare(X, Wc, bc, Win, Wout):
    """Host-side clustering, canonical layout, and per-core input maps."""
    X = np.asarray(X, np.float32)
    Wc = np.asarray(Wc, np.float32)
    bc = np.asarray(bc, np.float32)
    Win = np.asarray(Win, np.float32)
    Wout = np.asarray(Wout, np.float32)

    assign_all = np.stack(
        [(X[b] @ Wc.T + bc).argmax(-1) for b in range(B)]
    )  # [B, T]
    T2, units, denom_cols, order, Ws = make_schedule(assign_all)

    per_batch = []
    poss = []
    for b in range(B):
        a = assign_all[b]
        X2 = np.zeros((T2, D), np.float32)
        cid = np.full(T2, -1, np.int64)  # cluster id per column (-1 = pad)
        pos = np.empty(T, np.int64)  # original token -> column
        patch = np.empty((C, LH, 64), np.float32)
        for s in range(C):
            c = order[b, s]
            toks = np.nonzero(a == c)[0]
            n = len(toks)
            A = denom_cols[s]
            cid[A] = c  # denominator token
            patch[s] = float(T - n)
            cid[A + 1 : A + 1 + n] = c
            X2[A + 1 : A + 1 + n] = X[b, toks]
            pos[toks] = np.arange(A + 1, A + 1 + n)
        msl = SQL * (cid[None, :] == np.arange(C)[:, None]).astype(np.float32)
        per_batch.append(
            {
                "xt": np.ascontiguousarray(X2.T),
                "msl": np.ascontiguousarray(msl),
                "vpatch": patch,
            }
        )
        poss.append(pos)

    per_half = []
    for hh in range(2):
        r = slice(hh * 256, (hh + 1) * 256)
        wqkv = np.concatenate(
            [Win[0:D][r].T, Win[D : 2 * D][r].T, Win[2 * D :][r].T], axis=1
        )
        per_half.append(
            {
                "wqkv": np.ascontiguousarray(wqkv),
                "wo": np.ascontiguousarray(Wout[:, r].T),
            }
        )

    in_maps = [dict(per_batch[g // 2], **per_half[g % 2]) for g in range(8)]
    return (T2, units, denom_cols), in_maps, poss


_NC_CACHE = {}


def kernel(X, Wc, bc, Win, bin_, Wout, bout):
    assert not np.any(np.asarray(bin_)), "kernel assumes zero in_proj bias"
    sched, in_maps, poss = prepare(X, Wc, bc, Win, Wout)
    key = (sched[0], tuple(sched[1]), tuple(sched[2]))
    if key not in _NC_CACHE:
        _NC_CACHE[key] = build_nc(*sched)
    nc = _NC_CACHE[key]
    res = run_bass_kernel_spmd(nc, in_maps, core_ids=list(range(8)))
    outs = res.results
    bout = np.asarray(bout, np.float32)
    out = np.empty((B, T, D), np.float32)
    for b in range(B):
        full = outs[2 * b]["outT"] + outs[2 * b + 1]["outT"]  # [D, T2]
        out[b] = full.T[poss[b]] + bout
    return out


# revision 59
# speedup vs baseline: 1.2607x; 1.1056x over previous
"""Trainium2 Bass kernel for NeuralClusteringAttention.

Problem: B=4, T=1024, D=512, C=8 clusters, H=8 heads, fp32.
Reference: per-token cluster assignment (argmax of a linear projection), then
for each cluster c: full MHA over X*mask_c, output masked and summed over c.

Key algebraic collapse (headroom source): since every token belongs to exactly
one cluster and in_proj bias is zero, the C-pass reference reduces to ONE
masked-attention pass:
    out_i = Wout @ (sum_{j: c_j=c_i} e^{s_ij} v_j) / (sum_{j: c_j=c_i} e^{s_ij}
            + (T - n_{c_i})) + bout
where s_ij = q_i.k_j/sqrt(hd). The (T - n_c) term accounts for the e^0=1
contributions of masked keys in the reference softmax (masked scores are 0,
not -inf).

Implementation tricks:
- Cluster masking is folded into the QK^T contraction: it is augmented with
  sqrt(lambda)*one_hot(cluster) rows so intra-cluster pairs get +lambda;
  exp(s/8 + lambda/8*(E-1)) suppresses cross-cluster pairs to ~e^-24.
- Tokens are SORTED by cluster on the host, so attention only computes the
  near-block-diagonal tiles (the lambda-mask handles block-boundary overlap
  exactly). The per-batch layouts share one canonical slot structure so a
  single SPMD program serves all cores.
- The softmax denominator is produced by the PV matmul itself: V is augmented
  with 64 ones-columns (mass lands replicated on out-partitions 64..127), and
  one zero-X "denominator token" per cluster carries weight (T - n_c) in its
  ones-columns, so no separate correction pass is needed.
- Matmul operands use dtype float32r (TF32-like fast PE path, ~4x fp32).

Sharding: 8 cores = (4 batches) x (2 head-halves of 4 heads each). Each core
computes QKV projections for its 4 heads, masked attention, and a partial
output projection; host sums the two partials per batch and adds bout.
"""

import numpy as np

import concourse.bacc as bacc
import concourse.bass as bass
import concourse.mybir as mybir
import concourse.tile as tile
from concourse.bass_utils import run_bass_kernel_spmd

B, T, D, C, H = 4, 1024, 512, 8, 8
HD = D // H          # 64
LH = 4               # local heads per core
LAMBDA = 256.0       # cluster-mask additive bias (lambda/8 = 32 in exp domain)
SQL = 16.0           # sqrt(LAMBDA)
F32 = mybir.dt.float32
MMDT = mybir.dt.float32r  # matmul operand dtype (float32r = fast PE path)
CHUNK = 256          # attention i-chunk width (psum: [128, 4, 256] = 2 banks)


def make_schedule(assign_all):
    """Canonical cluster-slot layout shared by all batches (single SPMD prog).

    Returns (T2, units, denom_cols, slot_of, As, Ws):
      units: list of (c0, c1, jb0, jb1) attention work items
      denom_cols: canonical column of each slot's denominator token
    """
    counts = np.stack([np.bincount(a, minlength=C) for a in assign_all])  # [B,C]
    order = np.argsort(-counts, axis=1, kind="stable")  # [B, C] slot -> cluster
    sizes_sorted = -np.sort(-counts, axis=1)
    Ws = sizes_sorted.max(axis=0) + 1  # +1 denominator token per slot
    As = np.zeros(C, np.int64)
    As[1:] = np.cumsum(Ws)[:-1]
    used = int(As[-1] + Ws[-1])
    T2 = ((used + 127) // 128) * 128
    units = []
    for c0 in range(0, used, CHUNK):
        # float32r matmuls need even moving-dim; round widths to 4 columns
        # (pad columns hold zeros / inert masks, outputs there are dropped)
        c1 = min(c0 + CHUNK, ((used + 3) // 4) * 4)
        s_lo = int(np.searchsorted(As, c0, "right") - 1)
        s_hi = int(np.searchsorted(As, c1 - 1, "right") - 1)
        jb0 = int(As[s_lo]) // 128
        jb1 = (int(As[s_hi] + Ws[s_hi]) + 127) // 128
        jb1 = min(jb1, T2 // 128)
        units.append((c0, c1, jb0, jb1))
    return T2, units, [int(a) for a in As], order, Ws


def _kernel_body(tc, T2, units, denom_cols):
    nc = tc.nc
    NB = T2 // 128
    NSC = [(o, min(o + 512, T2)) for o in range(0, T2, 512)]

    xt_d = nc.dram_tensor("xt", [D, T2], F32, kind="ExternalInput").ap()
    wqkv_d = nc.dram_tensor("wqkv", [D, 3 * 256], F32, kind="ExternalInput").ap()
    wo_d = nc.dram_tensor("wo", [256, D], F32, kind="ExternalInput").ap()
    msl_d = nc.dram_tensor("msl", [C, T2], F32, kind="ExternalInput").ap()
    vp_d = nc.dram_tensor("vpatch", [C, LH, 64], F32, kind="ExternalInput").ap()
    out_d = nc.dram_tensor("outT", [D, T2], F32, kind="ExternalOutput").ap()

    with (
        tc.tile_pool(name="const", bufs=1) as const,
        tc.tile_pool(name="p1", bufs=8) as ppool,
        tc.tile_pool(name="rb", bufs=4) as rbpool,
        tc.tile_pool(name="acc", bufs=2, space="PSUM") as acc,
        tc.tile_pool(name="sps", bufs=2, space="PSUM") as sps,
    ):
        # ---- persistent SBUF tiles + input DMAs ----
        wqkv = const.tile([128, D // 128, 3 * 256], MMDT)
        wqkv_r = wqkv_d.rearrange("(c p) n -> p c n", p=128).bitcast(MMDT)
        xt = const.tile([128, D // 128, T2], MMDT)
        xt_r = xt_d.rearrange("(c p) t -> p c t", p=128).bitcast(MMDT)
        # split input DMA across both HWDGE issuing engines (SP + ACT) so the
        # two streams transfer concurrently: xt on ACT, weights on SP
        for dc in range(D // 128):
            nc.sync.dma_start(wqkv[:, dc, :], wqkv_r[:, dc, :])
            n0, n1 = NSC[0]
            nc.scalar.dma_start(xt[:, dc, n0:n1], xt_r[:, dc, n0:n1])
        for n0, n1 in NSC[1:]:
            for dc in range(D // 128):
                nc.scalar.dma_start(xt[:, dc, n0:n1], xt_r[:, dc, n0:n1])
        qtm = const.tile([72, LH, T2], MMDT)
        ktm = const.tile([72, LH, T2], MMDT)
        for h in range(LH):
            nc.sync.dma_start(qtm[64:72, h, :], msl_d[:, :].bitcast(MMDT))
            nc.sync.dma_start(ktm[64:72, h, :], msl_d[:, :].bitcast(MMDT))
        wo = const.tile([128, 2, D], MMDT)
        nc.sync.dma_start(
            wo[:], wo_d.rearrange("(c p) n -> p c n", p=128).bitcast(MMDT)
        )
        # V augmented with 64 ones-columns (softmax mass on partitions 64..127);
        # denominator-token rows carry (T - n_c) instead of 1.
        v_sb = const.tile([128, NB, LH, 128], MMDT)
        nc.vector.memset(v_sb[:, :, :, 64:128].bitcast(F32), 1.0)
        for s, dc_col in enumerate(denom_cols):
            nc.sync.dma_start(
                v_sb[dc_col % 128 : dc_col % 128 + 1, dc_col // 128, :, 64:128],
                vp_d[s : s + 1, :, :].bitcast(MMDT),
            )
        biasm = const.tile([128, 1], F32)
        nc.vector.memset(biasm[:, :], -LAMBDA / 8.0)
        o_sc = const.tile([128, 2, T2], MMDT)
        used = units[-1][1]
        if used < T2:
            nc.vector.memset(o_sc[:, :, used:T2].bitcast(F32), 0.0)

        # ---- QKV projections (head pairs, transposed layouts) ----
        pi = 0
        for hp in range(2):
            for w_off, dst in ((0, qtm), (256, ktm)):
                for n0, n1 in NSC:
                    # proj runs before any attention accumulator exists, so it
                    # can borrow the idle acc-pool banks for double pipelining
                    pool_ = (sps, acc)[pi % 2]
                    pi += 1
                    ps = pool_.tile(
                        [128, 512], F32, tag=("sps", "acc")[(pi - 1) % 2],
                        name=f"ps{pi}",
                    )
                    for dc in range(D // 128):
                        nc.tensor.matmul(
                            ps[:, : n1 - n0],
                            lhsT=wqkv[:, dc, w_off + hp * 128 : w_off + (hp + 1) * 128],
                            rhs=xt[:, dc, n0:n1],
                            start=(dc == 0),
                            stop=(dc == D // 128 - 1),
                        )
                    # even head partition-aligned -> ACT (idle in this phase);
                    # odd head cross-half -> DVE (64-wide cross-quadrant move)
                    nc.scalar.copy(dst[0:64, 2 * hp, n0:n1], ps[0:64, : n1 - n0])
                    nc.vector.tensor_copy(
                        dst[0:64, 2 * hp + 1, n0:n1], ps[64:128, : n1 - n0]
                    )

        # ---- V projection in natural [token, dim] layout, all 4 heads ----
        for tb in range(NB):
            pool_ = (sps, acc)[tb % 2]
            psv = pool_.tile(
                [128, 256], F32, tag=("sps", "acc")[tb % 2], name=f"psv{tb}"
            )
            for dc in range(D // 128):
                nc.tensor.matmul(
                    psv[:, 0:256],
                    lhsT=xt[:, dc, tb * 128 : (tb + 1) * 128],
                    rhs=wqkv[:, dc, 512:768],
                    start=(dc == 0),
                    stop=(dc == D // 128 - 1),
                )
            nc.vector.tensor_copy(
                v_sb[:, tb, :, 0:64],
                psv[:, 0:256].rearrange("p (h d) -> p h d", h=4),
            )

        # ---- block-sparse masked attention (4 heads batched per unit) ----
        # PSUM start=True zeroes a whole 2KB bank, so the two heads sharing a
        # bank form ONE accumulation group: even head starts, odd head stops.
        for ui, (c0, c1, jb0, jb1) in enumerate(units):
            W = c1 - c0
            # fixed 256 stride keeps head regions at exact half-bank offsets
            oaug = acc.tile([128, LH, CHUNK], F32, tag="acc", name=f"oaug{ui}")
            for jb in range(jb0, jb1):
                s_ps = sps.tile(
                    [128, LH, CHUNK], F32, tag="sps", name=f"sps{ui}_{jb}"
                )
                for h in range(LH):
                    nc.tensor.matmul(
                        s_ps[:, h, 0:W],
                        lhsT=ktm[0:72, h, jb * 128 : (jb + 1) * 128],
                        rhs=qtm[0:72, h, c0:c1],
                        start=(h % 2 == 0),
                        stop=(h % 2 == 1),
                    )
                p1 = ppool.tile(
                    [128, LH, CHUNK], MMDT, tag="p1", name=f"p1_{ui}_{jb}"
                )
                nc.scalar.activation(
                    p1[:, :, 0:W],
                    s_ps[:, :, 0:W],
                    mybir.ActivationFunctionType.Exp,
                    bias=biasm[:, :],
                    scale=0.125,
                )
                for h in range(LH):
                    nc.tensor.matmul(
                        oaug[:, h, 0:W],
                        lhsT=v_sb[:, jb, h, 0:128],
                        rhs=p1[:, h, 0:W],
                        start=(h % 2 == 0 and jb == jb0),
                        stop=(h % 2 == 1 and jb == jb1 - 1),
                    )
            recip = rbpool.tile([64, LH, CHUNK], F32, tag="recip", name=f"rc{ui}")
            nc.vector.reciprocal(recip[:, :, 0:W], oaug[64:128, :, 0:W])
            for h in range(LH):
                nc.vector.tensor_mul(
                    o_sc[64 * (h % 2) : 64 * (h % 2) + 64, h // 2, c0:c1],
                    oaug[0:64, h, 0:W],
                    recip[:, h, 0:W],
                )



        # ---- output projection (partial over this core's heads) ----
        for doc in range(4):
            for n0, n1 in NSC:
                po = sps.tile([128, 512], F32, tag="sps", name=f"po{doc}_{n0}")
                for dhc in range(2):
                    nc.tensor.matmul(
                        po[:, : n1 - n0],
                        lhsT=wo[:, dhc, doc * 128 : (doc + 1) * 128],
                        rhs=o_sc[:, dhc, n0:n1],
                        start=(dhc == 0),
                        stop=(dhc == 1),
                    )
                ost = rbpool.tile([128, 512], F32, tag="ost", name=f"ost{doc}_{n0}")
                nc.scalar.copy(ost[:, : n1 - n0], po[:, : n1 - n0])
                nc.sync.dma_start(
                    out_d[doc * 128 : (doc + 1) * 128, n0:n1], ost[:, : n1 - n0]
                )


def build_nc(T2, units, denom_cols):
    nc = bacc.Bacc("TRN2", target_bir_lowering=False, debug=False, num_devices=8)
    with tile.TileContext(nc) as tc:
        _kernel_body(tc, T2, units, denom_cols)
    nc.compile()
    return nc


def prepare(X, Wc, bc, Win, Wout):
    """Host-side clustering, canonical layout, and per-core input maps."""
    X = np.asarray(X, np.float32)
    Wc = np.asarray(Wc, np.float32)
    bc = np.asarray(bc, np.float32)
    Win = np.asarray(Win, np.float32)
    Wout = np.asarray(Wout, np.float32)

    assign_all = np.stack(
        [(X[b] @ Wc.T + bc).argmax(-1) for b in range(B)]
    )  # [B, T]
    T2, units, denom_cols, order, Ws = make_schedule(assign_all)

    per_batch = []
    poss = []
    for b in range(B):
        a = assign_all[b]
        X2 = np.zeros((T2, D), np.float32)
        cid = np.full(T2, -1, np.int64)  # cluster id per column (-1 = pad)
        pos = np.empty(T, np.int64)  # original token -> column
        patch = np.empty((C, LH, 64), np.float32)
        for s in range(C):
            c = order[b, s]
            toks = np.nonzero(a == c)[0]
            n = len(toks)
            A = denom_cols[s]
            cid[A] = c  # denominator token
            patch[s] = float(T - n)
            cid[A + 1 : A + 1 + n] = c
            X2[A + 1 : A + 1 + n] = X[b, toks]
            pos[toks] = np.arange(A + 1, A + 1 + n)
        msl = SQL * (cid[None, :] == np.arange(C)[:, None]).astype(np.float32)
        per_batch.append(
            {
                "xt": np.ascontiguousarray(X2.T),
                "msl": np.ascontiguousarray(msl),
                "vpatch": patch,
            }
        )
        poss.append(pos)

    per_half = []
    for hh in range(2):
        r = slice(hh * 256, (hh + 1) * 256)
        wqkv = np.concatenate(
            [Win[0:D][r].T, Win[D : 2 * D][r].T, Win[2 * D :][r].T], axis=1
        )
        per_half.append(
            {
                "wqkv": np.ascontiguousarray(wqkv),
                "wo": np.ascontiguousarray(Wout[:, r].T),
            }
        )

    in_maps = [dict(per_batch[g // 2], **per_half[g % 2]) for g in range(8)]
    return (T2, units, denom_cols), in_maps, poss


_NC_CACHE = {}


def kernel(X, Wc, bc, Win, bin_, Wout, bout):
    assert not np.any(np.asarray(bin_)), "kernel assumes zero in_proj bias"
    sched, in_maps, poss = prepare(X, Wc, bc, Win, Wout)
    key = (sched[0], tuple(sched[1]), tuple(sched[2]))
    if key not in _NC_CACHE:
        _NC_CACHE[key] = build_nc(*sched)
    nc = _NC_CACHE[key]
    res = run_bass_kernel_spmd(nc, in_maps, core_ids=list(range(8)))
    outs = res.results
    bout = np.asarray(bout, np.float32)
    out = np.empty((B, T, D), np.float32)
    for b in range(B):
        full = outs[2 * b]["outT"] + outs[2 * b + 1]["outT"]  # [D, T2]
        out[b] = full.T[poss[b]] + bout
    return out


# revision 62
# speedup vs baseline: 1.3102x; 1.0393x over previous
"""Trainium2 Bass kernel for NeuralClusteringAttention.

Problem: B=4, T=1024, D=512, C=8 clusters, H=8 heads, fp32.
Reference: per-token cluster assignment (argmax of a linear projection), then
for each cluster c: full MHA over X*mask_c, output masked and summed over c.

Key algebraic collapse (headroom source): since every token belongs to exactly
one cluster and in_proj bias is zero, the C-pass reference reduces to ONE
masked-attention pass:
    out_i = Wout @ (sum_{j: c_j=c_i} e^{s_ij} v_j) / (sum_{j: c_j=c_i} e^{s_ij}
            + (T - n_{c_i})) + bout
where s_ij = q_i.k_j/sqrt(hd). The (T - n_c) term accounts for the e^0=1
contributions of masked keys in the reference softmax (masked scores are 0,
not -inf).

Implementation tricks:
- Cluster masking is folded into the QK^T contraction: it is augmented with
  sqrt(lambda)*one_hot(cluster) rows so intra-cluster pairs get +lambda;
  exp(s/8 + lambda/8*(E-1)) suppresses cross-cluster pairs to ~e^-24.
- Tokens are SORTED by cluster on the host, so attention only computes the
  near-block-diagonal tiles (the lambda-mask handles block-boundary overlap
  exactly). The per-batch layouts share one canonical slot structure so a
  single SPMD program serves all cores.
- The softmax denominator is produced by the PV matmul itself: V is augmented
  with 64 ones-columns (mass lands replicated on out-partitions 64..127), and
  one zero-X "denominator token" per cluster carries weight (T - n_c) in its
  ones-columns, so no separate correction pass is needed.
- Matmul operands use dtype float32r (TF32-like fast PE path, ~4x fp32).

Sharding: 8 cores = (4 batches) x (2 head-halves of 4 heads each). Each core
computes QKV projections for its 4 heads, masked attention, and a partial
output projection; host sums the two partials per batch and adds bout.
"""

import numpy as np

import concourse.bacc as bacc
import concourse.bass as bass
import concourse.mybir as mybir
import concourse.tile as tile
from concourse.bass_utils import run_bass_kernel_spmd

B, T, D, C, H = 4, 1024, 512, 8, 8
HD = D // H          # 64
LH = 4               # local heads per core
LAMBDA = 256.0       # cluster-mask additive bias (lambda/8 = 32 in exp domain)
SQL = 16.0           # sqrt(LAMBDA)
F32 = mybir.dt.float32
MMDT = mybir.dt.float32r  # matmul operand dtype (float32r = fast PE path)
CHUNK = 256          # attention i-chunk width (psum: [128, 4, 256] = 2 banks)


def make_schedule(assign_all):
    """Canonical cluster-slot layout shared by all batches (single SPMD prog).

    Returns (T2, units, denom_cols, slot_of, As, Ws):
      units: list of (c0, c1, jb0, jb1) attention work items
      denom_cols: canonical column of each slot's denominator token
    """
    counts = np.stack([np.bincount(a, minlength=C) for a in assign_all])  # [B,C]
    rank_order = np.argsort(-counts, axis=1, kind="stable")  # [B,C] rank->cluster
    sizes_sorted = -np.sort(-counts, axis=1)
    Ws0 = sizes_sorted.max(axis=0) + 1  # +1 denominator token per size-rank

    # Slot order is free: pick the permutation of canonical widths that
    # minimizes the attention exp-chain cost (sum over chunk-units and key
    # blocks of the ACT activation cycles).
    import itertools

    def act_cost(ws):
        As_ = np.zeros(C, np.int64)
        As_[1:] = np.cumsum(ws)[:-1]
        used_ = int(As_[-1] + ws[-1])
        T2_ = ((used_ + 127) // 128) * 128
        tot = 0
        for c0 in range(0, used_, CHUNK):
            c1 = min(c0 + CHUNK, ((used_ + 3) // 4) * 4)
            s_lo = int(np.searchsorted(As_, c0, "right") - 1)
            s_hi = int(np.searchsorted(As_, c1 - 1, "right") - 1)
            jb0 = int(As_[s_lo]) // 128
            jb1 = min((int(As_[s_hi] + ws[s_hi]) + 127) // 128, T2_ // 128)
            tot += (jb1 - jb0) * (4 * (c1 - c0) + 352)
        return tot

    best_perm = min(
        itertools.permutations(range(C)),
        key=lambda p: act_cost(Ws0[list(p)]),
    )
    Ws = Ws0[list(best_perm)]
    order = rank_order[:, list(best_perm)]  # [B, C] slot -> cluster
    As = np.zeros(C, np.int64)
    As[1:] = np.cumsum(Ws)[:-1]
    used = int(As[-1] + Ws[-1])
    T2 = ((used + 127) // 128) * 128
    units = []
    for c0 in range(0, used, CHUNK):
        # float32r matmuls need even moving-dim; round widths to 4 columns
        # (pad columns hold zeros / inert masks, outputs there are dropped)
        c1 = min(c0 + CHUNK, ((used + 3) // 4) * 4)
        s_lo = int(np.searchsorted(As, c0, "right") - 1)
        s_hi = int(np.searchsorted(As, c1 - 1, "right") - 1)
        jb0 = int(As[s_lo]) // 128
        jb1 = (int(As[s_hi] + Ws[s_hi]) + 127) // 128
        jb1 = min(jb1, T2 // 128)
        units.append((c0, c1, jb0, jb1))
    return T2, units, [int(a) for a in As], order, Ws


def _kernel_body(tc, T2, units, denom_cols):
    nc = tc.nc
    NB = T2 // 128
    NSC = [(o, min(o + 512, T2)) for o in range(0, T2, 512)]

    xt_d = nc.dram_tensor("xt", [D, T2], F32, kind="ExternalInput").ap()
    wqkv_d = nc.dram_tensor("wqkv", [D, 3 * 256], F32, kind="ExternalInput").ap()
    wo_d = nc.dram_tensor("wo", [256, D], F32, kind="ExternalInput").ap()
    msl_d = nc.dram_tensor("msl", [C, T2], F32, kind="ExternalInput").ap()
    vp_d = nc.dram_tensor("vpatch", [C, LH, 64], F32, kind="ExternalInput").ap()
    out_d = nc.dram_tensor("outT", [D, T2], F32, kind="ExternalOutput").ap()

    with (
        tc.tile_pool(name="const", bufs=1) as const,
        tc.tile_pool(name="p1", bufs=8) as ppool,
        tc.tile_pool(name="rb", bufs=4) as rbpool,
        tc.tile_pool(name="acc", bufs=2, space="PSUM") as acc,
        tc.tile_pool(name="sps", bufs=2, space="PSUM") as sps,
    ):
        # ---- persistent SBUF tiles + input DMAs ----
        wqkv = const.tile([128, D // 128, 3 * 256], MMDT)
        wqkv_r = wqkv_d.rearrange("(c p) n -> p c n", p=128).bitcast(MMDT)
        xt = const.tile([128, D // 128, T2], MMDT)
        xt_r = xt_d.rearrange("(c p) t -> p c t", p=128).bitcast(MMDT)
        # split input DMA across both HWDGE issuing engines (SP + ACT) so the
        # two streams transfer concurrently: xt on ACT, weights on SP
        for dc in range(D // 128):
            nc.sync.dma_start(wqkv[:, dc, :], wqkv_r[:, dc, :])
            n0, n1 = NSC[0]
            nc.scalar.dma_start(xt[:, dc, n0:n1], xt_r[:, dc, n0:n1])
        for n0, n1 in NSC[1:]:
            for dc in range(D // 128):
                nc.scalar.dma_start(xt[:, dc, n0:n1], xt_r[:, dc, n0:n1])
        qtm = const.tile([72, LH, T2], MMDT)
        ktm = const.tile([72, LH, T2], MMDT)
        for h in range(LH):
            nc.sync.dma_start(qtm[64:72, h, :], msl_d[:, :].bitcast(MMDT))
            nc.sync.dma_start(ktm[64:72, h, :], msl_d[:, :].bitcast(MMDT))
        wo = const.tile([128, 2, D], MMDT)
        nc.sync.dma_start(
            wo[:], wo_d.rearrange("(c p) n -> p c n", p=128).bitcast(MMDT)
        )
        # V augmented with 64 ones-columns (softmax mass on partitions 64..127);
        # denominator-token rows carry (T - n_c) instead of 1.
        v_sb = const.tile([128, NB, LH, 128], MMDT)
        nc.vector.memset(v_sb[:, :, :, 64:128].bitcast(F32), 1.0)
        for s, dc_col in enumerate(denom_cols):
            nc.sync.dma_start(
                v_sb[dc_col % 128 : dc_col % 128 + 1, dc_col // 128, :, 64:128],
                vp_d[s : s + 1, :, :].bitcast(MMDT),
            )
        biasm = const.tile([128, 1], F32)
        nc.vector.memset(biasm[:, :], -LAMBDA / 8.0)
        o_sc = const.tile([128, 2, T2], MMDT)
        used = units[-1][1]
        if used < T2:
            nc.vector.memset(o_sc[:, :, used:T2].bitcast(F32), 0.0)

        # ---- QKV projections (head pairs, transposed layouts) ----
        pi = 0
        for hp in range(2):
            for w_off, dst in ((0, qtm), (256, ktm)):
                for n0, n1 in NSC:
                    # proj runs before any attention accumulator exists, so it
                    # can borrow the idle acc-pool banks for double pipelining
                    pool_ = (sps, acc)[pi % 2]
                    pi += 1
                    ps = pool_.tile(
                        [128, 512], F32, tag=("sps", "acc")[(pi - 1) % 2],
                        name=f"ps{pi}",
                    )
                    for dc in range(D // 128):
                        nc.tensor.matmul(
                            ps[:, : n1 - n0],
                            lhsT=wqkv[:, dc, w_off + hp * 128 : w_off + (hp + 1) * 128],
                            rhs=xt[:, dc, n0:n1],
                            start=(dc == 0),
                            stop=(dc == D // 128 - 1),
                        )
                    # even head partition-aligned -> ACT (idle in this phase);
                    # odd head cross-half -> DVE (64-wide cross-quadrant move)
                    nc.scalar.copy(dst[0:64, 2 * hp, n0:n1], ps[0:64, : n1 - n0])
                    nc.vector.tensor_copy(
                        dst[0:64, 2 * hp + 1, n0:n1], ps[64:128, : n1 - n0]
                    )

        # ---- V projection in natural [token, dim] layout, all 4 heads ----
        for tb in range(NB):
            pool_ = (sps, acc)[tb % 2]
            psv = pool_.tile(
                [128, 256], F32, tag=("sps", "acc")[tb % 2], name=f"psv{tb}"
            )
            for dc in range(D // 128):
                nc.tensor.matmul(
                    psv[:, 0:256],
                    lhsT=xt[:, dc, tb * 128 : (tb + 1) * 128],
                    rhs=wqkv[:, dc, 512:768],
                    start=(dc == 0),
                    stop=(dc == D // 128 - 1),
                )
            nc.vector.tensor_copy(
                v_sb[:, tb, :, 0:64],
                psv[:, 0:256].rearrange("p (h d) -> p h d", h=4),
            )

        # ---- block-sparse masked attention (4 heads batched per unit) ----
        # PSUM start=True zeroes a whole 2KB bank, so the two heads sharing a
        # bank form ONE accumulation group: even head starts, odd head stops.
        for ui, (c0, c1, jb0, jb1) in enumerate(units):
            W = c1 - c0
            # fixed 256 stride keeps head regions at exact half-bank offsets
            oaug = acc.tile([128, LH, CHUNK], F32, tag="acc", name=f"oaug{ui}")
            for jb in range(jb0, jb1):
                s_ps = sps.tile(
                    [128, LH, CHUNK], F32, tag="sps", name=f"sps{ui}_{jb}"
                )
                for h in range(LH):
                    nc.tensor.matmul(
                        s_ps[:, h, 0:W],
                        lhsT=ktm[0:72, h, jb * 128 : (jb + 1) * 128],
                        rhs=qtm[0:72, h, c0:c1],
                        start=(h % 2 == 0),
                        stop=(h % 2 == 1),
                    )
                p1 = ppool.tile(
                    [128, LH, CHUNK], MMDT, tag="p1", name=f"p1_{ui}_{jb}"
                )
                nc.scalar.activation(
                    p1[:, :, 0:W],
                    s_ps[:, :, 0:W],
                    mybir.ActivationFunctionType.Exp,
                    bias=biasm[:, :],
                    scale=0.125,
                )
                for h in range(LH):
                    nc.tensor.matmul(
                        oaug[:, h, 0:W],
                        lhsT=v_sb[:, jb, h, 0:128],
                        rhs=p1[:, h, 0:W],
                        start=(h % 2 == 0 and jb == jb0),
                        stop=(h % 2 == 1 and jb == jb1 - 1),
                    )
            recip = rbpool.tile([64, LH, CHUNK], F32, tag="recip", name=f"rc{ui}")
            nc.vector.reciprocal(recip[:, :, 0:W], oaug[64:128, :, 0:W])
            for h in range(LH):
                nc.vector.tensor_mul(
                    o_sc[64 * (h % 2) : 64 * (h % 2) + 64, h // 2, c0:c1],
                    oaug[0:64, h, 0:W],
                    recip[:, h, 0:W],
                )



        # ---- output projection (partial over this core's heads) ----
        for doc in range(4):
            for n0, n1 in NSC:
                po = sps.tile([128, 512], F32, tag="sps", name=f"po{doc}_{n0}")
                for dhc in range(2):
                    nc.tensor.matmul(
                        po[:, : n1 - n0],
                        lhsT=wo[:, dhc, doc * 128 : (doc + 1) * 128],
                        rhs=o_sc[:, dhc, n0:n1],
                        start=(dhc == 0),
                        stop=(dhc == 1),
                    )
                ost = rbpool.tile([128, 512], F32, tag="ost", name=f"ost{doc}_{n0}")
                nc.scalar.copy(ost[:, : n1 - n0], po[:, : n1 - n0])
                nc.sync.dma_start(
                    out_d[doc * 128 : (doc + 1) * 128, n0:n1], ost[:, : n1 - n0]
                )


def build_nc(T2, units, denom_cols):
    nc = bacc.Bacc("TRN2", target_bir_lowering=False, debug=False, num_devices=8)
    with tile.TileContext(nc) as tc:
        _kernel_body(tc, T2, units, denom_cols)
    nc.compile()
    return nc


def prepare(X, Wc, bc, Win, Wout):
    """Host-side clustering, canonical layout, and per-core input maps."""
    X = np.asarray(X, np.float32)
    Wc = np.asarray(Wc, np.float32)
    bc = np.asarray(bc, np.float32)
    Win = np.asarray(Win, np.float32)
    Wout = np.asarray(Wout, np.float32)

    assign_all = np.stack(
        [(X[b] @ Wc.T + bc).argmax(-1) for b in range(B)]
    )  # [B, T]
    T2, units, denom_cols, order, Ws = make_schedule(assign_all)

    per_batch = []
    poss = []
    for b in range(B):
        a = assign_all[b]
        X2 = np.zeros((T2, D), np.float32)
        cid = np.full(T2, -1, np.int64)  # cluster id per column (-1 = pad)
        pos = np.empty(T, np.int64)  # original token -> column
        patch = np.empty((C, LH, 64), np.float32)
        for s in range(C):
            c = order[b, s]
            toks = np.nonzero(a == c)[0]
            n = len(toks)
            A = denom_cols[s]
            cid[A] = c  # denominator token
            patch[s] = float(T - n)
            cid[A + 1 : A + 1 + n] = c
            X2[A + 1 : A + 1 + n] = X[b, toks]
            pos[toks] = np.arange(A + 1, A + 1 + n)
        msl = SQL * (cid[None, :] == np.arange(C)[:, None]).astype(np.float32)
        per_batch.append(
            {
                "xt": np.ascontiguousarray(X2.T),
                "msl": np.ascontiguousarray(msl),
                "vpatch": patch,
            }
        )
        poss.append(pos)

    per_half = []
    for hh in range(2):
        r = slice(hh * 256, (hh + 1) * 256)
        wqkv = np.concatenate(
            [Win[0:D][r].T, Win[D : 2 * D][r].T, Win[2 * D :][r].T], axis=1
        )
        per_half.append(
            {
                "wqkv": np.ascontiguousarray(wqkv),
                "wo": np.ascontiguousarray(Wout[:, r].T),
            }
        )

    in_maps = [dict(per_batch[g // 2], **per_half[g % 2]) for g in range(8)]
    return (T2, units, denom_cols), in_maps, poss


_NC_CACHE = {}


def kernel(X, Wc, bc, Win, bin_, Wout, bout):
    assert not np.any(np.asarray(bin_)), "kernel assumes zero in_proj bias"
    sched, in_maps, poss = prepare(X, Wc, bc, Win, Wout)
    key = (sched[0], tuple(sched[1]), tuple(sched[2]))
    if key not in _NC_CACHE:
        _NC_CACHE[key] = build_nc(*sched)
    nc = _NC_CACHE[key]
    res = run_bass_kernel_spmd(nc, in_maps, core_ids=list(range(8)))
    outs = res.results
    bout = np.asarray(bout, np.float32)
    out = np.empty((B, T, D), np.float32)
    for b in range(B):
        full = outs[2 * b]["outT"] + outs[2 * b + 1]["outT"]  # [D, T2]
        out[b] = full.T[poss[b]] + bout
    return out


# revision 64
# speedup vs baseline: 1.3106x; 1.0003x over previous
"""Trainium2 Bass kernel for NeuralClusteringAttention.

Problem: B=4, T=1024, D=512, C=8 clusters, H=8 heads, fp32.
Reference: per-token cluster assignment (argmax of a linear projection), then
for each cluster c: full MHA over X*mask_c, output masked and summed over c.

Key algebraic collapse (headroom source): since every token belongs to exactly
one cluster and in_proj bias is zero, the C-pass reference reduces to ONE
masked-attention pass:
    out_i = Wout @ (sum_{j: c_j=c_i} e^{s_ij} v_j) / (sum_{j: c_j=c_i} e^{s_ij}
            + (T - n_{c_i})) + bout
where s_ij = q_i.k_j/sqrt(hd). The (T - n_c) term accounts for the e^0=1
contributions of masked keys in the reference softmax (masked scores are 0,
not -inf).

Implementation tricks:
- Cluster masking is folded into the QK^T contraction: it is augmented with
  sqrt(lambda)*one_hot(cluster) rows so intra-cluster pairs get +lambda;
  exp(s/8 + lambda/8*(E-1)) suppresses cross-cluster pairs to ~e^-24.
- Tokens are SORTED by cluster on the host, so attention only computes the
  near-block-diagonal tiles (the lambda-mask handles block-boundary overlap
  exactly). The per-batch layouts share one canonical slot structure so a
  single SPMD program serves all cores.
- The softmax denominator is produced by the PV matmul itself: V is augmented
  with 64 ones-columns (mass lands replicated on out-partitions 64..127), and
  one zero-X "denominator token" per cluster carries weight (T - n_c) in its
  ones-columns, so no separate correction pass is needed.
- Matmul operands use dtype float32r (TF32-like fast PE path, ~4x fp32).

Sharding: 8 cores = (4 batches) x (2 head-halves of 4 heads each). Each core
computes QKV projections for its 4 heads, masked attention, and a partial
output projection; host sums the two partials per batch and adds bout.
"""

import numpy as np

import concourse.bacc as bacc
import concourse.bass as bass
import concourse.mybir as mybir
import concourse.tile as tile
from concourse.bass_utils import run_bass_kernel_spmd

B, T, D, C, H = 4, 1024, 512, 8, 8
HD = D // H          # 64
LH = 4               # local heads per core
LAMBDA = 256.0       # cluster-mask additive bias (lambda/8 = 32 in exp domain)
SQL = 16.0           # sqrt(LAMBDA)
F32 = mybir.dt.float32
MMDT = mybir.dt.float32r  # matmul operand dtype (float32r = fast PE path)
CHUNK = 256          # attention i-chunk width (psum: [128, 4, 256] = 2 banks)


def make_schedule(assign_all):
    """Canonical cluster-slot layout shared by all batches (single SPMD prog).

    Returns (T2, units, denom_cols, slot_of, As, Ws):
      units: list of (c0, c1, jb0, jb1) attention work items
      denom_cols: canonical column of each slot's denominator token
    """
    counts = np.stack([np.bincount(a, minlength=C) for a in assign_all])  # [B,C]
    rank_order = np.argsort(-counts, axis=1, kind="stable")  # [B,C] rank->cluster
    sizes_sorted = -np.sort(-counts, axis=1)
    Ws0 = sizes_sorted.max(axis=0) + 1  # +1 denominator token per size-rank

    # Slot order is free: pick the permutation of canonical widths that
    # minimizes the attention exp-chain cost (sum over chunk-units and key
    # blocks of the ACT activation cycles).
    import itertools

    def act_cost(ws):
        As_ = np.zeros(C, np.int64)
        As_[1:] = np.cumsum(ws)[:-1]
        used_ = int(As_[-1] + ws[-1])
        T2_ = ((used_ + 127) // 128) * 128
        tot = 0
        for c0 in range(0, used_, CHUNK):
            c1 = min(c0 + CHUNK, ((used_ + 3) // 4) * 4)
            s_lo = int(np.searchsorted(As_, c0, "right") - 1)
            s_hi = int(np.searchsorted(As_, c1 - 1, "right") - 1)
            jb0 = int(As_[s_lo]) // 128
            jb1 = min((int(As_[s_hi] + ws[s_hi]) + 127) // 128, T2_ // 128)
            tot += (jb1 - jb0) * (4 * (c1 - c0) + 352)
        return tot

    best_perm = min(
        itertools.permutations(range(C)),
        key=lambda p: act_cost(Ws0[list(p)]),
    )
    Ws = Ws0[list(best_perm)]
    order = rank_order[:, list(best_perm)]  # [B, C] slot -> cluster
    As = np.zeros(C, np.int64)
    As[1:] = np.cumsum(Ws)[:-1]
    used = int(As[-1] + Ws[-1])
    T2 = ((used + 127) // 128) * 128
    units = []
    for c0 in range(0, used, CHUNK):
        # float32r matmuls need even moving-dim; round widths to 4 columns
        # (pad columns hold zeros / inert masks, outputs there are dropped)
        c1 = min(c0 + CHUNK, ((used + 3) // 4) * 4)
        s_lo = int(np.searchsorted(As, c0, "right") - 1)
        s_hi = int(np.searchsorted(As, c1 - 1, "right") - 1)
        jb0 = int(As[s_lo]) // 128
        jb1 = (int(As[s_hi] + Ws[s_hi]) + 127) // 128
        jb1 = min(jb1, T2 // 128)
        units.append((c0, c1, jb0, jb1))
    return T2, units, [int(a) for a in As], order, Ws


def _kernel_body(tc, T2, units, denom_cols):
    nc = tc.nc
    NB = T2 // 128
    NSC = [(o, min(o + 512, T2)) for o in range(0, T2, 512)]

    xt_d = nc.dram_tensor("xt", [D, T2], F32, kind="ExternalInput").ap()
    wqkv_d = nc.dram_tensor("wqkv", [D, 3 * 256], F32, kind="ExternalInput").ap()
    wo_d = nc.dram_tensor("wo", [256, D], F32, kind="ExternalInput").ap()
    msl_d = nc.dram_tensor("msl", [C, T2], F32, kind="ExternalInput").ap()
    vp_d = nc.dram_tensor("vpatch", [C, LH, 64], F32, kind="ExternalInput").ap()
    out_d = nc.dram_tensor("outT", [D, T2], F32, kind="ExternalOutput").ap()

    with (
        tc.tile_pool(name="const", bufs=1) as const,
        tc.tile_pool(name="p1", bufs=8) as ppool,
        tc.tile_pool(name="rb", bufs=4) as rbpool,
        tc.tile_pool(name="acc", bufs=2, space="PSUM") as acc,
        tc.tile_pool(name="sps", bufs=2, space="PSUM") as sps,
    ):
        # ---- persistent SBUF tiles + input DMAs ----
        wqkv = const.tile([128, D // 128, 3 * 256], MMDT)
        wqkv_r = wqkv_d.rearrange("(c p) n -> p c n", p=128).bitcast(MMDT)
        xt = const.tile([128, D // 128, T2], MMDT)
        xt_r = xt_d.rearrange("(c p) t -> p c t", p=128).bitcast(MMDT)
        # split input DMA across both HWDGE issuing engines (SP + ACT) so the
        # two streams transfer concurrently: xt on ACT, weights on SP
        for dc in range(D // 128):
            nc.sync.dma_start(wqkv[:, dc, :], wqkv_r[:, dc, :])
            n0, n1 = NSC[0]
            nc.scalar.dma_start(xt[:, dc, n0:n1], xt_r[:, dc, n0:n1])
        for n0, n1 in NSC[1:]:
            for dc in range(D // 128):
                nc.scalar.dma_start(xt[:, dc, n0:n1], xt_r[:, dc, n0:n1])
        qtm = const.tile([72, LH, T2], MMDT)
        ktm = const.tile([72, LH, T2], MMDT)
        for h in range(LH):
            nc.sync.dma_start(qtm[64:72, h, :], msl_d[:, :].bitcast(MMDT))
            nc.sync.dma_start(ktm[64:72, h, :], msl_d[:, :].bitcast(MMDT))
        wo = const.tile([128, 2, D], MMDT)
        nc.sync.dma_start(
            wo[:], wo_d.rearrange("(c p) n -> p c n", p=128).bitcast(MMDT)
        )
        # V augmented with 64 ones-columns (softmax mass on partitions 64..127);
        # denominator-token rows carry (T - n_c) instead of 1.
        v_sb = const.tile([128, NB, LH, 128], MMDT)
        nc.vector.memset(v_sb[:, :, :, 64:128].bitcast(F32), 1.0)
        for s, dc_col in enumerate(denom_cols):
            nc.sync.dma_start(
                v_sb[dc_col % 128 : dc_col % 128 + 1, dc_col // 128, :, 64:128],
                vp_d[s : s + 1, :, :].bitcast(MMDT),
            )
        biasm = const.tile([128, 1], F32)
        nc.vector.memset(biasm[:, :], -LAMBDA / 8.0)
        o_sc = const.tile([128, 2, T2], MMDT)
        used = units[-1][1]
        if used < T2:
            nc.vector.memset(o_sc[:, :, used:T2].bitcast(F32), 0.0)

        # ---- QKV projections (head pairs, transposed layouts) ----
        pi = 0
        for hp in range(2):
            for w_off, dst in ((0, qtm), (256, ktm)):
                for n0, n1 in NSC:
                    # proj runs before any attention accumulator exists, so it
                    # can borrow the idle acc-pool banks for double pipelining
                    pool_ = (sps, acc)[pi % 2]
                    pi += 1
                    ps = pool_.tile(
                        [128, 512], F32, tag=("sps", "acc")[(pi - 1) % 2],
                        name=f"ps{pi}",
                    )
                    for dc in range(D // 128):
                        nc.tensor.matmul(
                            ps[:, : n1 - n0],
                            lhsT=wqkv[:, dc, w_off + hp * 128 : w_off + (hp + 1) * 128],
                            rhs=xt[:, dc, n0:n1],
                            start=(dc == 0),
                            stop=(dc == D // 128 - 1),
                        )
                    # even head partition-aligned -> ACT (idle in this phase);
                    # odd head cross-half -> DVE (64-wide cross-quadrant move)
                    nc.scalar.copy(dst[0:64, 2 * hp, n0:n1], ps[0:64, : n1 - n0])
                    nc.vector.tensor_copy(
                        dst[0:64, 2 * hp + 1, n0:n1], ps[64:128, : n1 - n0]
                    )

        # ---- V projection in natural [token, dim] layout, all 4 heads ----
        for tb in range(NB):
            pool_ = (sps, acc)[tb % 2]
            psv = pool_.tile(
                [128, 256], F32, tag=("sps", "acc")[tb % 2], name=f"psv{tb}"
            )
            for dc in range(D // 128):
                nc.tensor.matmul(
                    psv[:, 0:256],
                    lhsT=xt[:, dc, tb * 128 : (tb + 1) * 128],
                    rhs=wqkv[:, dc, 512:768],
                    start=(dc == 0),
                    stop=(dc == D // 128 - 1),
                )
            cp = nc.scalar.copy if tb % 2 else nc.vector.tensor_copy
            cp(
                v_sb[:, tb, :, 0:64],
                psv[:, 0:256].rearrange("p (h d) -> p h d", h=4),
            )

        # ---- block-sparse masked attention (4 heads batched per unit) ----
        # PSUM start=True zeroes a whole 2KB bank, so the two heads sharing a
        # bank form ONE accumulation group: even head starts, odd head stops.
        for ui, (c0, c1, jb0, jb1) in enumerate(units):
            W = c1 - c0
            # fixed 256 stride keeps head regions at exact half-bank offsets
            oaug = acc.tile([128, LH, CHUNK], F32, tag="acc", name=f"oaug{ui}")
            for jb in range(jb0, jb1):
                s_ps = sps.tile(
                    [128, LH, CHUNK], F32, tag="sps", name=f"sps{ui}_{jb}"
                )
                for h in range(LH):
                    nc.tensor.matmul(
                        s_ps[:, h, 0:W],
                        lhsT=ktm[0:72, h, jb * 128 : (jb + 1) * 128],
                        rhs=qtm[0:72, h, c0:c1],
                        start=(h % 2 == 0),
                        stop=(h % 2 == 1),
                    )
                p1 = ppool.tile(
                    [128, LH, CHUNK], MMDT, tag="p1", name=f"p1_{ui}_{jb}"
                )
                nc.scalar.activation(
                    p1[:, :, 0:W],
                    s_ps[:, :, 0:W],
                    mybir.ActivationFunctionType.Exp,
                    bias=biasm[:, :],
                    scale=0.125,
                )
                for h in range(LH):
                    nc.tensor.matmul(
                        oaug[:, h, 0:W],
                        lhsT=v_sb[:, jb, h, 0:128],
                        rhs=p1[:, h, 0:W],
                        start=(h % 2 == 0 and jb == jb0),
                        stop=(h % 2 == 1 and jb == jb1 - 1),
                    )
            recip = rbpool.tile([64, LH, CHUNK], F32, tag="recip", name=f"rc{ui}")
            nc.vector.reciprocal(recip[:, :, 0:W], oaug[64:128, :, 0:W])
            for h in range(LH):
                nc.vector.tensor_mul(
                    o_sc[64 * (h % 2) : 64 * (h % 2) + 64, h // 2, c0:c1],
                    oaug[0:64, h, 0:W],
                    recip[:, h, 0:W],
                )



        # ---- output projection (partial over this core's heads) ----
        for doc in range(4):
            for n0, n1 in NSC:
                po = sps.tile([128, 512], F32, tag="sps", name=f"po{doc}_{n0}")
                for dhc in range(2):
                    nc.tensor.matmul(
                        po[:, : n1 - n0],
                        lhsT=wo[:, dhc, doc * 128 : (doc + 1) * 128],
                        rhs=o_sc[:, dhc, n0:n1],
                        start=(dhc == 0),
                        stop=(dhc == 1),
                    )
                ost = rbpool.tile([128, 512], F32, tag="ost", name=f"ost{doc}_{n0}")
                nc.scalar.copy(ost[:, : n1 - n0], po[:, : n1 - n0])
                nc.sync.dma_start(
                    out_d[doc * 128 : (doc + 1) * 128, n0:n1], ost[:, : n1 - n0]
                )


def build_nc(T2, units, denom_cols):
    nc = bacc.Bacc("TRN2", target_bir_lowering=False, debug=False, num_devices=8)
    with tile.TileContext(nc) as tc:
        _kernel_body(tc, T2, units, denom_cols)
    nc.compile()
    return nc


def prepare(X, Wc, bc, Win, Wout):
    """Host-side clustering, canonical layout, and per-core input maps."""
    X = np.asarray(X, np.float32)
    Wc = np.asarray(Wc, np.float32)
    bc = np.asarray(bc, np.float32)
    Win = np.asarray(Win, np.float32)
    Wout = np.asarray(Wout, np.float32)

    assign_all = np.stack(
        [(X[b] @ Wc.T + bc).argmax(-1) for b in range(B)]
    )  # [B, T]
    T2, units, denom_cols, order, Ws = make_schedule(assign_all)

    per_batch = []
    poss = []
    for b in range(B):
        a = assign_all[b]
        X2 = np.zeros((T2, D), np.float32)
        cid = np.full(T2, -1, np.int64)  # cluster id per column (-1 = pad)
        pos = np.empty(T, np.int64)  # original token -> column
        patch = np.empty((C, LH, 64), np.float32)
        for s in range(C):
            c = order[b, s]
            toks = np.nonzero(a == c)[0]
            n = len(toks)
            A = denom_cols[s]
            cid[A] = c  # denominator token
            patch[s] = float(T - n)
            cid[A + 1 : A + 1 + n] = c
            X2[A + 1 : A + 1 + n] = X[b, toks]
            pos[toks] = np.arange(A + 1, A + 1 + n)
        msl = SQL * (cid[None, :] == np.arange(C)[:, None]).astype(np.float32)
        per_batch.append(
            {
                "xt": np.ascontiguousarray(X2.T),
                "msl": np.ascontiguousarray(msl),
                "vpatch": patch,
            }
        )
        poss.append(pos)

    per_half = []
    for hh in range(2):
        r = slice(hh * 256, (hh + 1) * 256)
        wqkv = np.concatenate(
            [Win[0:D][r].T, Win[D : 2 * D][r].T, Win[2 * D :][r].T], axis=1
        )
        per_half.append(
            {
                "wqkv": np.ascontiguousarray(wqkv),
                "wo": np.ascontiguousarray(Wout[:, r].T),
            }
        )

    in_maps = [dict(per_batch[g // 2], **per_half[g % 2]) for g in range(8)]
    return (T2, units, denom_cols), in_maps, poss


_NC_CACHE = {}


def kernel(X, Wc, bc, Win, bin_, Wout, bout):
    assert not np.any(np.asarray(bin_)), "kernel assumes zero in_proj bias"
    sched, in_maps, poss = prepare(X, Wc, bc, Win, Wout)
    key = (sched[0], tuple(sched[1]), tuple(sched[2]))
    if key not in _NC_CACHE:
        _NC_CACHE[key] = build_nc(*sched)
    nc = _NC_CACHE[key]
    res = run_bass_kernel_spmd(nc, in_maps, core_ids=list(range(8)))
    outs = res.results
    bout = np.asarray(bout, np.float32)
    out = np.empty((B, T, D), np.float32)
    for b in range(B):
        full = outs[2 * b]["outT"] + outs[2 * b + 1]["outT"]  # [D, T2]
        out[b] = full.T[poss[b]] + bout
    return out


# revision 74
# speedup vs baseline: 1.3552x; 1.0340x over previous
"""Trainium2 Bass kernel for NeuralClusteringAttention.

Problem: B=4, T=1024, D=512, C=8 clusters, H=8 heads, fp32.
Reference: per-token cluster assignment (argmax of a linear projection), then
for each cluster c: full MHA over X*mask_c, output masked and summed over c.

Key algebraic collapse (headroom source): since every token belongs to exactly
one cluster and in_proj bias is zero, the C-pass reference reduces to ONE
masked-attention pass:
    out_i = Wout @ (sum_{j: c_j=c_i} e^{s_ij} v_j) / (sum_{j: c_j=c_i} e^{s_ij}
            + (T - n_{c_i})) + bout
where s_ij = q_i.k_j/sqrt(hd). The (T - n_c) term accounts for the e^0=1
contributions of masked keys in the reference softmax (masked scores are 0,
not -inf).

Implementation tricks:
- Cluster masking is folded into the QK^T contraction: it is augmented with
  sqrt(lambda)*one_hot(cluster) rows so intra-cluster pairs get +lambda;
  exp(s/8 + lambda/8*(E-1)) suppresses cross-cluster pairs to ~e^-24.
- Tokens are SORTED by cluster on the host, so attention only computes the
  near-block-diagonal tiles (the lambda-mask handles block-boundary overlap
  exactly). The per-batch layouts share one canonical slot structure so a
  single SPMD program serves all cores.
- The softmax denominator is produced by the PV matmul itself: V is augmented
  with 64 ones-columns (mass lands replicated on out-partitions 64..127), and
  one zero-X "denominator token" per cluster carries weight (T - n_c) in its
  ones-columns, so no separate correction pass is needed.
- Matmul operands use dtype float32r (TF32-like fast PE path, ~4x fp32).

Sharding: 8 cores = (4 batches) x (2 head-halves of 4 heads each). Each core
computes QKV projections for its 4 heads, masked attention, and a partial
output projection; host sums the two partials per batch and adds bout.
"""

import numpy as np

import concourse.bacc as bacc
import concourse.bass as bass
import concourse.mybir as mybir
import concourse.tile as tile
from concourse.bass_utils import run_bass_kernel_spmd

B, T, D, C, H = 4, 1024, 512, 8, 8
HD = D // H          # 64
LH = 4               # local heads per core
LAMBDA = 256.0       # cluster-mask additive bias (lambda/8 = 32 in exp domain)
SQL = 16.0           # sqrt(LAMBDA)
F32 = mybir.dt.float32
MMDT = mybir.dt.float32r  # matmul operand dtype (float32r = fast PE path)
CHUNK = 256          # attention i-chunk width (psum: [128, 4, 256] = 2 banks)


def make_schedule(assign_all):
    """Canonical cluster-slot layout shared by all batches (single SPMD prog).

    Returns (T2, units, denom_cols, slot_of, As, Ws):
      units: list of (c0, c1, jb0, jb1) attention work items
      denom_cols: canonical column of each slot's denominator token
    """
    counts = np.stack([np.bincount(a, minlength=C) for a in assign_all])  # [B,C]
    rank_order = np.argsort(-counts, axis=1, kind="stable")  # [B,C] rank->cluster
    sizes_sorted = -np.sort(-counts, axis=1)
    Ws0 = sizes_sorted.max(axis=0) + 1  # +1 denominator token per size-rank

    # Slot order is free: pick the permutation of canonical widths that
    # minimizes the attention exp-chain cost (sum over chunk-units and key
    # blocks of the ACT activation cycles).
    import itertools

    def act_cost(ws):
        As_ = np.zeros(C, np.int64)
        As_[1:] = np.cumsum(ws)[:-1]
        used_ = int(As_[-1] + ws[-1])
        T2_ = ((used_ + 127) // 128) * 128
        tot = 0
        for c0 in range(0, used_, CHUNK):
            c1 = min(c0 + CHUNK, ((used_ + 3) // 4) * 4)
            s_lo = int(np.searchsorted(As_, c0, "right") - 1)
            s_hi = int(np.searchsorted(As_, c1 - 1, "right") - 1)
            jb0 = int(As_[s_lo]) // 128
            jb1 = min((int(As_[s_hi] + ws[s_hi]) + 127) // 128, T2_ // 128)
            tot += (jb1 - jb0) * (4 * (c1 - c0) + 352)
        return tot

    best_perm = min(
        itertools.permutations(range(C)),
        key=lambda p: act_cost(Ws0[list(p)]),
    )
    Ws = Ws0[list(best_perm)]
    order = rank_order[:, list(best_perm)]  # [B, C] slot -> cluster
    As = np.zeros(C, np.int64)
    As[1:] = np.cumsum(Ws)[:-1]
    used = int(As[-1] + Ws[-1])
    T2 = ((used + 127) // 128) * 128
    units = []
    for c0 in range(0, used, CHUNK):
        # float32r matmuls need even moving-dim; round widths to 4 columns
        # (pad columns hold zeros / inert masks, outputs there are dropped)
        c1 = min(c0 + CHUNK, ((used + 3) // 4) * 4)
        s_lo = int(np.searchsorted(As, c0, "right") - 1)
        s_hi = int(np.searchsorted(As, c1 - 1, "right") - 1)
        jb0 = int(As[s_lo]) // 128
        jb1 = (int(As[s_hi] + Ws[s_hi]) + 127) // 128
        jb1 = min(jb1, T2 // 128)
        units.append((c0, c1, jb0, jb1))
    return T2, units, [int(a) for a in As], order, Ws


def _kernel_body(tc, T2, units, denom_cols):
    nc = tc.nc
    NB = T2 // 128
    NSC = [(o, min(o + 512, T2)) for o in range(0, T2, 512)]

    xt_d = nc.dram_tensor("xt", [D, T2], F32, kind="ExternalInput").ap()
    wqkv_d = nc.dram_tensor("wqkv", [D, 3 * 256], F32, kind="ExternalInput").ap()
    wo_d = nc.dram_tensor("wo", [256, D], F32, kind="ExternalInput").ap()
    msl_d = nc.dram_tensor("msl", [C, T2], F32, kind="ExternalInput").ap()
    vp_d = nc.dram_tensor("vpatch", [C, LH, 64], F32, kind="ExternalInput").ap()
    out_d = nc.dram_tensor("outT", [D, T2], F32, kind="ExternalOutput").ap()

    with (
        tc.tile_pool(name="const", bufs=1) as const,
        tc.tile_pool(name="p1", bufs=8) as ppool,
        tc.tile_pool(name="rb", bufs=4) as rbpool,
        tc.tile_pool(name="acc", bufs=2, space="PSUM") as acc,
        tc.tile_pool(name="sps", bufs=2, space="PSUM") as sps,
    ):
        # ---- persistent SBUF tiles + input DMAs ----
        wqkv = const.tile([128, D // 128, 3 * 256], MMDT)
        wqkv_r = wqkv_d.rearrange("(c p) n -> p c n", p=128).bitcast(MMDT)
        xt = const.tile([128, D // 128, T2], MMDT)
        xt_r = xt_d.rearrange("(c p) t -> p c t", p=128).bitcast(MMDT)
        # split input DMA across both HWDGE issuing engines (SP + ACT) so the
        # two streams transfer concurrently: xt on ACT, weights on SP
        for dc in range(D // 128):
            nc.sync.dma_start(wqkv[:, dc, :], wqkv_r[:, dc, :])
            n0, n1 = NSC[0]
            nc.scalar.dma_start(xt[:, dc, n0:n1], xt_r[:, dc, n0:n1])
        for n0, n1 in NSC[1:]:
            for dc in range(D // 128):
                nc.scalar.dma_start(xt[:, dc, n0:n1], xt_r[:, dc, n0:n1])
        qtm = const.tile([72, LH, T2], MMDT)
        ktm = const.tile([72, LH, T2], MMDT)
        for h in range(LH):
            nc.sync.dma_start(qtm[64:72, h, :], msl_d[:, :].bitcast(MMDT))
            nc.sync.dma_start(ktm[64:72, h, :], msl_d[:, :].bitcast(MMDT))
        wo = const.tile([128, 2, D], MMDT)
        nc.sync.dma_start(
            wo[:], wo_d.rearrange("(c p) n -> p c n", p=128).bitcast(MMDT)
        )
        # V augmented with 64 ones-columns (softmax mass on partitions 64..127);
        # denominator-token rows carry (T - n_c) instead of 1.
        v_sb = const.tile([128, NB, LH, 128], MMDT)
        nc.vector.memset(v_sb[:, :, :, 64:128].bitcast(F32), 1.0)
        for s, dc_col in enumerate(denom_cols):
            nc.sync.dma_start(
                v_sb[dc_col % 128 : dc_col % 128 + 1, dc_col // 128, :, 64:128],
                vp_d[s : s + 1, :, :].bitcast(MMDT),
            )
        biasm = const.tile([128, 1], F32)
        nc.vector.memset(biasm[:, :], -LAMBDA / 8.0)
        o_sc = const.tile([128, 2, T2], MMDT)
        used = units[-1][1]
        if used < T2:
            nc.vector.memset(o_sc[:, :, used:T2].bitcast(F32), 0.0)

        # ---- QKV projections (head pairs, transposed layouts) ----
        pi = 0
        for hp in range(2):
            for w_off, dst in ((0, qtm), (256, ktm)):
                for n0, n1 in NSC:
                    # proj runs before any attention accumulator exists, so it
                    # can borrow the idle acc-pool banks for double pipelining
                    pool_ = (sps, acc)[pi % 2]
                    pi += 1
                    ps = pool_.tile(
                        [128, 512], F32, tag=("sps", "acc")[(pi - 1) % 2],
                        name=f"ps{pi}",
                    )
                    for dc in range(D // 128):
                        nc.tensor.matmul(
                            ps[:, : n1 - n0],
                            lhsT=wqkv[:, dc, w_off + hp * 128 : w_off + (hp + 1) * 128],
                            rhs=xt[:, dc, n0:n1],
                            start=(dc == 0),
                            stop=(dc == D // 128 - 1),
                        )
                    # even head partition-aligned -> ACT (idle in this phase);
                    # odd head cross-half -> DVE (64-wide cross-quadrant move)
                    nc.scalar.copy(dst[0:64, 2 * hp, n0:n1], ps[0:64, : n1 - n0])
                    nc.vector.tensor_copy(
                        dst[0:64, 2 * hp + 1, n0:n1], ps[64:128, : n1 - n0]
                    )

        # ---- V projection in natural [token, dim] layout, all 4 heads ----
        for tb in range(NB):
            pool_ = (sps, acc)[tb % 2]
            psv = pool_.tile(
                [128, 256], F32, tag=("sps", "acc")[tb % 2], name=f"psv{tb}"
            )
            for dc in range(D // 128):
                nc.tensor.matmul(
                    psv[:, 0:256],
                    lhsT=xt[:, dc, tb * 128 : (tb + 1) * 128],
                    rhs=wqkv[:, dc, 512:768],
                    start=(dc == 0),
                    stop=(dc == D // 128 - 1),
                )
            cp = nc.scalar.copy if tb % 2 else nc.vector.tensor_copy
            cp(
                v_sb[:, tb, :, 0:64],
                psv[:, 0:256].rearrange("p (h d) -> p h d", h=4),
            )

        # ---- block-sparse masked attention (4 heads batched per unit) ----
        # PSUM start=True zeroes a whole 2KB bank, so the two heads sharing a
        # bank form ONE accumulation group: even head starts, odd head stops.
        for ui, (c0, c1, jb0, jb1) in enumerate(units):
            W = c1 - c0
            # fixed 256 stride keeps head regions at exact half-bank offsets
            oaug = acc.tile([128, LH, CHUNK], F32, tag="acc", name=f"oaug{ui}")
            for jb in range(jb0, jb1):
                s_ps = sps.tile(
                    [128, LH, CHUNK], F32, tag="sps", name=f"sps{ui}_{jb}"
                )
                for h in range(LH):
                    nc.tensor.matmul(
                        s_ps[:, h, 0:W],
                        lhsT=ktm[0:72, h, jb * 128 : (jb + 1) * 128],
                        rhs=qtm[0:72, h, c0:c1],
                        start=(h % 2 == 0),
                        stop=(h % 2 == 1),
                    )
                p1 = ppool.tile(
                    [128, LH, CHUNK], MMDT, tag="p1", name=f"p1_{ui}_{jb}"
                )
                nc.scalar.activation(
                    p1[:, :, 0:W],
                    s_ps[:, :, 0:W],
                    mybir.ActivationFunctionType.Exp,
                    bias=biasm[:, :],
                    scale=0.125,
                )
                for h in range(LH):
                    nc.tensor.matmul(
                        oaug[:, h, 0:W],
                        lhsT=v_sb[:, jb, h, 0:128],
                        rhs=p1[:, h, 0:W],
                        start=(h % 2 == 0 and jb == jb0),
                        stop=(h % 2 == 1 and jb == jb1 - 1),
                    )
            recip = rbpool.tile([64, LH, CHUNK], F32, tag="recip", name=f"rc{ui}")
            nc.vector.reciprocal(recip[:, :, 0:W], oaug[64:128, :, 0:W])
            for h in range(LH):
                nc.vector.tensor_mul(
                    o_sc[64 * (h % 2) : 64 * (h % 2) + 64, h // 2, c0:c1],
                    oaug[0:64, h, 0:W],
                    recip[:, h, 0:W],
                )



        # ---- output projection (partial over this core's heads) ----
        for n0, n1 in NSC:
            for doc in range(4):
                po = sps.tile([128, 512], F32, tag="sps", name=f"po{doc}_{n0}")
                for dhc in range(2):
                    nc.tensor.matmul(
                        po[:, : n1 - n0],
                        lhsT=wo[:, dhc, doc * 128 : (doc + 1) * 128],
                        rhs=o_sc[:, dhc, n0:n1],
                        start=(dhc == 0),
                        stop=(dhc == 1),
                    )
                ost = rbpool.tile([128, 512], F32, tag="ost", name=f"ost{doc}_{n0}")
                nc.scalar.copy(ost[:, : n1 - n0], po[:, : n1 - n0])
                nc.sync.dma_start(
                    out_d[doc * 128 : (doc + 1) * 128, n0:n1], ost[:, : n1 - n0]
                )


def build_nc(T2, units, denom_cols):
    nc = bacc.Bacc("TRN2", target_bir_lowering=False, debug=False, num_devices=8)
    with tile.TileContext(nc) as tc:
        _kernel_body(tc, T2, units, denom_cols)
    nc.compile()
    return nc


def prepare(X, Wc, bc, Win, Wout):
    """Host-side clustering, canonical layout, and per-core input maps."""
    X = np.asarray(X, np.float32)
    Wc = np.asarray(Wc, np.float32)
    bc = np.asarray(bc, np.float32)
    Win = np.asarray(Win, np.float32)
    Wout = np.asarray(Wout, np.float32)

    assign_all = np.stack(
        [(X[b] @ Wc.T + bc).argmax(-1) for b in range(B)]
    )  # [B, T]
    T2, units, denom_cols, order, Ws = make_schedule(assign_all)

    per_batch = []
    poss = []
    for b in range(B):
        a = assign_all[b]
        X2 = np.zeros((T2, D), np.float32)
        cid = np.full(T2, -1, np.int64)  # cluster id per column (-1 = pad)
        pos = np.empty(T, np.int64)  # original token -> column
        patch = np.empty((C, LH, 64), np.float32)
        for s in range(C):
            c = order[b, s]
            toks = np.nonzero(a == c)[0]
            n = len(toks)
            A = denom_cols[s]
            cid[A] = c  # denominator token
            patch[s] = float(T - n)
            cid[A + 1 : A + 1 + n] = c
            X2[A + 1 : A + 1 + n] = X[b, toks]
            pos[toks] = np.arange(A + 1, A + 1 + n)
        msl = SQL * (cid[None, :] == np.arange(C)[:, None]).astype(np.float32)
        per_batch.append(
            {
                "xt": np.ascontiguousarray(X2.T),
                "msl": np.ascontiguousarray(msl),
                "vpatch": patch,
            }
        )
        poss.append(pos)

    per_half = []
    for hh in range(2):
        r = slice(hh * 256, (hh + 1) * 256)
        wqkv = np.concatenate(
            [Win[0:D][r].T, Win[D : 2 * D][r].T, Win[2 * D :][r].T], axis=1
        )
        per_half.append(
            {
                "wqkv": np.ascontiguousarray(wqkv),
                "wo": np.ascontiguousarray(Wout[:, r].T),
            }
        )

    in_maps = [dict(per_batch[g // 2], **per_half[g % 2]) for g in range(8)]
    return (T2, units, denom_cols), in_maps, poss


_NC_CACHE = {}


def kernel(X, Wc, bc, Win, bin_, Wout, bout):
    assert not np.any(np.asarray(bin_)), "kernel assumes zero in_proj bias"
    sched, in_maps, poss = prepare(X, Wc, bc, Win, Wout)
    key = (sched[0], tuple(sched[1]), tuple(sched[2]))
    if key not in _NC_CACHE:
        _NC_CACHE[key] = build_nc(*sched)
    nc = _NC_CACHE[key]
    res = run_bass_kernel_spmd(nc, in_maps, core_ids=list(range(8)))
    outs = res.results
    bout = np.asarray(bout, np.float32)
    out = np.empty((B, T, D), np.float32)
    for b in range(B):
        full = outs[2 * b]["outT"] + outs[2 * b + 1]["outT"]  # [D, T2]
        out[b] = full.T[poss[b]] + bout
    return out
